# revision 33
# baseline (speedup 1.0000x reference)
"""AdaptiveConv2d Trainium2 kernel (full-array conv mapping), v4.

Reference computation (B=32, CIN=32, COUT=64, K=3, H=W=128, FIN=64):
    h   = relu(z @ w1.T + b1); h = relu(h @ w2.T + b2)
    aw  = relu(h @ w3.T + b3)                      # (B, 18496)
    kern = aw[:, :18432] -> (B, 64, 32, 3, 3)      # per-sample conv weights
    bias = aw[:, 18432:]                           # (B, 64)
    y = relu(conv2d_same(x, kern) + bias)          # (B, 64, 128, 128)

Strategy: pure data parallel over 8 NeuronCores, 4 samples per core.

Conv mapping (75% PE-cell utilization): contraction rows =
(dy in 0..4, ci in 0..32) = 128 full rows, where dy indexes the 4
consecutive padded-x rows covering one OUTPUT ROW PAIR; PE columns =
(b in {0,1}, cout), b selecting which output row of the pair.
lhsT[(dy,ci),(b,co)] = w[ky=dy-b, kx, ci, co] (zero outside
0<=dy-b<=2).  The three kx taps accumulate into one PSUM bank via rhs
column shifts.  Each (group-of-4-row-pairs, kx) is one full-width
512-column matmul (192 per core ~= 41.5us of streaming).  The conv
body (psum banks, epilogues, per-half-sample outputs) matches the
measured-best v1 structure.

Front-of-kernel (v1 conv started ~31us; v4 targets ~21-22us):
 - decoy DMAs absorb each ring's first-completion latency into
   corners of a tiny decoyT tile (not dumT: that serialized the PE
   warm-up behind an HBM receipt)
 - the MLP inputs (zaT/w1a/w2a) ride inside the FIRST w3 sync-ring
   chunk -- one large-DMA receipt instead of 3 small-DMA latencies
   (~4us) on the scalar ring
 - MLP final layer is ONE full-128 stationary (block-diagonal h2
   replica; w2a grows a [0..,1] column so the h2 matmul emits the
   1.0 bias-feature row directly) streaming all 4672 w3 columns in 10
   matmuls: one cheap hidden LDWEIGHTS each instead of 4 serialized
   32x32 quadrant loads per 512-col tile
 - awp gets 6 PSUM buffers (h1p/h2p in a sub-pool that closes first)
   and each chunk's relu is split scalar/vector half-and-half, so the
   jt stream doesn't throttle on relu drain (v2/v3 trapped the warm-up
   matmuls behind buffer-starved jts)
 - sample-0 rearranges all on the ScalarE HWDGE ring (v2 put b=1 on
   gpsimd SWDGE whose completion receipts cost ~3us more)
 - PE kept busy through the front (dummies between the tiny MLP
   layers) so HAM un-throttles before the w3 stream

x SBUF layout xq[s][dy*32+ci, j, c]: partitions 0:64 hold
xpad[ci, 2j+dy] (dy=0,1), partitions 64:128 the same shifted one row
pair (dy=2,3).  Duplication on HOST: a device-side dup lands on
64-partition destinations = 8 of 16 SBUF AXI ports, so it saves HBM
bytes but loses more on port-limited SBUF time + ring serialization.

Output staged bf16 (halves HBM write traffic; host upcasts); samples
0-2 drain as one 2MB DMA per sample (alternating scalar/gpsimd
queues; ~365 vs ~341 GB/s at 1MB), sample 3's first half as 1MB on
gpsimd and its final half as quarters on scalar+sync (HWDGE receipts
gate program end); epilogue (bias+relu, psum->bf16) alternates
scalar/vector per bank, except sample 0 all-vector (ScalarE is still
issuing sample-1 rearranges in that window).

Compute bf16, accumulate f32, output bf16. L2 rel err ~5e-3.
"""

import sys
import types

import numpy as np
import ml_dtypes

BF16 = ml_dtypes.bfloat16

B, CIN, COUT, KS, H, W, FIN = 32, 32, 64, 3, 128, 128, 64
L1, L2 = 20, 30
NKW = CIN * COUT * KS * KS  # 18432
NOUT = NKW + COUT  # 18496
N_CORES = 8
BS = B // N_CORES  # 4 samples per core
NCW = NKW // 4  # 4608 weight columns per cin-chunk
NCH = NCW + COUT  # 4672 including bias tail
XJ = 65  # j extent of xq (row pairs + 1 for the dup source)
XC = 132  # padded col extent (130 used)
N_DUMMY = 9  # PE warm-up matmuls bridging the prologue->MLP gap
N_DUMMY_MID = 3  # fillers between the tiny MLP layers (HAM busy-window)
N_DUMMY2 = 8  # post-MLP warm-up until the rearrange-paced fillers
MOF = 56  # packed-MLP column offset of the w3 chunks


def _install_ntff_hook():
    """Make run_bass_kernel_spmd(trace=True) work under axon by providing
    the antenv.axon_hooks module the image lacks. Safe no-op on failure."""
    try:
        if "antenv.axon_hooks" in sys.modules:
            return
        import antenv

        mod = types.ModuleType("antenv.axon_hooks")
        mod._hook = None
        mod.set_axon_ntff_profile_hook = lambda h: setattr(mod, "_hook", h)
        mod.get_axon_ntff_profile_hook = lambda: mod._hook
        sys.modules["antenv.axon_hooks"] = mod
        antenv.axon_hooks = mod
        from trn_agent_boot.trn_boot import _ntff_profile_via_ctypes

        hook = _ntff_profile_via_ctypes("/opt/axon/libaxon_pjrt.so")
        if hook is not None:
            mod.set_axon_ntff_profile_hook(hook)
    except Exception:
        pass


def build_nc():
    import concourse.tile as tile
    from concourse import bacc, mybir

    dt = mybir.dt
    Relu = mybir.ActivationFunctionType.Relu

    nc = bacc.Bacc(
        "TRN2", target_bir_lowering=False, debug=False, num_devices=1
    )
    # all samples: full 128-partition layout (host-duplicated top half)
    xa0 = nc.dram_tensor("xa0", [128, XJ, XC], dt.bfloat16, kind="ExternalInput")
    xa123 = nc.dram_tensor(
        "xa123", [BS - 1, 128, XJ, XC], dt.bfloat16, kind="ExternalInput"
    )
    # Packed weights+inputs tensor.  Cols 0:4 zaT (rows 0:65), 4:24 w1a
    # (rows 0:65), 24:55 w2a (rows 0:21, 31 cols), 55:MOF pad.  From col
    # MOF: w3 split into 4 column-chunks stacked on partitions: row
    # 32c+k is (w3.T row k) of chunk c for k<30, row 32c+30 is b3 of
    # chunk c, row 32c+31 is zeros.  Chunk c covers ci in [8c, 8c+8);
    # its 4608 cols are ordered (ky, ci8, kx, co).  Chunk 3 also
    # carries the 64 conv-bias cols at the tail (others zero-padded).
    w3a = nc.dram_tensor(
        "w3a", [128, MOF + NCH], dt.bfloat16, kind="ExternalInput"
    )
    # output: [s, (b,co), hs, gg, rp, c] bf16; y[s,co,64*hs+8*gg+2*rp+b,c]
    # (partition-major within a sample so a whole-sample DMA from the
    # [128, 2, 8, 4, W] staging tile is order-preserving)
    outd = nc.dram_tensor(
        "out", [BS, 128, 2, 8, 4, W], dt.bfloat16, kind="ExternalOutput"
    )

    with tile.TileContext(nc) as tc:
        with (
            tc.tile_pool(name="const", bufs=1) as cp,
            tc.tile_pool(name="outp", bufs=3) as op,
        ):
            xq = [
                cp.tile([128, XJ, XC], dt.bfloat16, name=f"xq{s}") for s in range(BS)
            ]
            w3s = cp.tile([128, MOF + NCH], dt.bfloat16)
            # generated weights: aw[32c+8s, ky*1536+ci8*192+kx*64+co]
            aw = cp.tile([128, NCH], dt.bfloat16)
            # conv lhsT staging: wq[(dy,ci), s, b, kx, co]
            wq = cp.tile([128, BS, 2, KS, COUT], dt.bfloat16)
            # fused lhsT: wqf[(dy,ci), s, kx, (b,co)] -- one matmul per
            # (group, kx) slot; matmul stationary APs allow only one
            # free dim, so the (b,co) interleave is an engine-copy hop
            wqf = cp.tile([128, BS, KS, 128], dt.bfloat16)
            zs = w3s[0 : FIN + 1, 0:BS]
            w1s = w3s[0 : FIN + 1, 4 : 4 + L1]
            w2s = w3s[0 : L1 + 1, 24 : 24 + L2 + 1]
            h1a = cp.tile([L1 + 1, BS], dt.bfloat16)
            # block-diagonal h2 replica: h2d[(c,k), 32c+8s] = h2[s,k],
            # rows 32c+30 = 1.0 at its block's sample cols, else 0.
            h2d = cp.tile([128, 128], dt.bfloat16)
            dumT = cp.tile([128, 512], dt.bfloat16)
            decoyT = cp.tile([2, 4], dt.bfloat16)
            # per-sample conv bias, transposed onto partitions: column 8s
            # of btT/biasF holds bias_s[co] at partition b*64+co (b both)
            btT = cp.tile([128, 32], dt.bfloat16)
            biasF = cp.tile([128, 32], dt.float32)

            # ---- input DMAs ----
            # sync ring, ordered by need: first chunk carries the packed
            # MLP inputs + the w3 ky0 block, then ky1/ky2 + bias tail,
            # sample 0 in two j-chunks, then samples 1-3.  The decoy
            # absorbs the ring's first-completion latency.
            nc.sync.dma_start(decoyT[0:1, 0:2], w3a.ap()[0:1, 0:2])
            # first chunk deliberately small (MLP inputs + jt0 block):
            # its completion receipt gates the whole h1->h2->aw chain
            wbounds = [0, MOF + 512, MOF + 1536, MOF + 3072, MOF + NCW, MOF + NCH]
            for n0, n1 in zip(wbounds[:-1], wbounds[1:]):
                nc.sync.dma_start(w3s[:, n0:n1], w3a.ap()[:, n0:n1])
            JH = 33
            nc.sync.dma_start(xq[0][:, 0:JH, :], xa0.ap()[:, 0:JH, :])
            nc.sync.dma_start(xq[0][:, JH:XJ, :], xa0.ap()[:, JH:XJ, :])
            for s in range(1, BS):
                nc.sync.dma_start(xq[s][:, :, :], xa123.ap()[s - 1])
            # scalar-ring decoy: pre-warm the ACT ring so the rearrange
            # DMAs (its first real traffic) skip the first-receipt cost
            nc.scalar.dma_start(decoyT[1:2, 0:2], w3a.ap()[0:1, 0:2])
            # pre-warm the gpsimd SWDGE path
            nc.gpsimd.dma_start(decoyT[0:1, 2:3], w3a.ap()[0:1, 0:1])

            # ---- early memsets ----
            nc.vector.memset(dumT[:], 0.0)
            nc.vector.memset(h1a[:], 1.0)
            nc.vector.memset(h2d[:], 0.0)
            # only the never-DMA-written corner blocks of wq need zeroing
            nc.gpsimd.memset(wq[96:128, :, 0, :, :], 0.0)
            nc.gpsimd.memset(wq[0:32, :, 1, :, :], 0.0)

            # ---- PE warm-up (bridge prologue -> MLP/w3 arrival) ----
            with tc.tile_pool(name="dummp", bufs=1, space="PSUM") as dmp:
                dup = dmp.tile([128, 512], dt.float32)
                for i in range(N_DUMMY):
                    nc.tensor.matmul(
                        dup[:], dumT[:, 0:128], dumT[:], start=(i == 0), stop=False
                    )

                # ---- MLP generating conv weights ----
                with tc.tile_pool(name="mlp12", bufs=1, space="PSUM") as m12:
                    h1p = m12.tile([L1, BS], dt.float32)
                    nc.tensor.matmul(h1p[:], w1s, zs, start=True, stop=True)
                    nc.scalar.activation(h1a[0:L1, :], h1p[:], Relu)
                    for i in range(N_DUMMY_MID):
                        nc.tensor.matmul(
                            dup[:], dumT[:, 0:128], dumT[:], start=False, stop=False
                        )

                    h2p = m12.tile([L2 + 1, BS], dt.float32)
                    nc.tensor.matmul(h2p[:], w2s, h1a[:], start=True, stop=True)
                    for i in range(N_DUMMY_MID):
                        nc.tensor.matmul(
                            dup[:], dumT[:, 0:128], dumT[:], start=False, stop=False
                        )
                    # h2 (+ its constant-1.0 bias row) -> 4 diagonal
                    # blocks (sample s in column 32c+8s)
                    for c in range(4):
                        dst = h2d[
                            32 * c : 32 * c + L2 + 1, 32 * c : 32 * c + 8 * BS : 8
                        ]
                        if c % 2 == 0:
                            nc.scalar.activation(dst, h2p[:], Relu)
                        else:
                            nc.vector.tensor_scalar_max(dst, h2p[:], 0.0)

                # final layer: one 128-wide stationary (block-diag h2),
                # stream all w3 columns.  Each chunk's relu is split
                # half/half scalar+vector so the drain keeps pace with
                # the matmuls (awp has 6 buffers).
                with tc.tile_pool(name="mlpp", bufs=1, space="PSUM") as mp:
                    ntile = (NCH + 511) // 512
                    for jt in range(ntile):
                        n0 = jt * 512
                        n1 = min(NCH, n0 + 512)
                        awp = mp.tile(
                            [128, n1 - n0],
                            dt.float32,
                            tag="awp",
                            bufs=6,
                            name="awp",
                        )
                        nc.tensor.matmul(
                            awp[:],
                            h2d[:],
                            w3s[:, MOF + n0 : MOF + n1],
                            start=True,
                            stop=True,
                        )
                        if jt == ntile - 1:
                            nc.scalar.activation(aw[:, n0:n1], awp[:], Relu)
                            # bias -> partitions via DVE 32x32 block
                            # transposes (co halves x b halves), then one
                            # f32 convert.  No DMA involved.
                            for bb in range(2):
                                for ch in range(2):
                                    nc.vector.transpose(
                                        btT[
                                            64 * bb + 32 * ch : 64 * bb + 32 * ch + 32,
                                            :,
                                        ],
                                        aw[96:128, NCW + 32 * ch : NCW + 32 * ch + 32],
                                    )
                            nc.vector.tensor_copy(biasF[:], btT[:])
                            continue
                        # 60/40 vector/scalar: ScalarE is the scarce
                        # engine here (it must also issue the sample-0
                        # rearranges right after)
                        mid = n0 + 320
                        nc.vector.tensor_scalar_max(
                            aw[:, n0:mid], awp[:, 0 : mid - n0], 0.0
                        )
                        nc.scalar.activation(
                            aw[:, mid:n1], awp[:, mid - n0 : n1 - n0], Relu
                        )

                # ---- second warm-up block: keep the PE busy (HAM warm)
                # until the rearrange-paced fillers below take over
                for i in range(N_DUMMY2):
                    nc.tensor.matmul(
                        dup[:], dumT[:, 0:128], dumT[:], start=False, stop=False
                    )

                # ---- rearrange generated weights into the block lhsT --
                # per (s, b, ky): one partition-expanding SBUF->SBUF DMA
                # of one 32-partition block (all ci) of one ky tap into
                # wq partitions 32*(b+ky).  The SEQUENCER ISSUE time
                # (~0.5us per DMA) is the critical path, so spread:
                # sample 0 on the ScalarE HWDGE ring, sample 1 on the
                # sync ring (its data queues behind the x stream --
                # fine, its deadline is a sample-conv later), samples
                # 2-3 on gpsimd SWDGE.
                def rearr(eng, s, b, ky):
                    p0 = 32 * (b + ky)
                    n0 = ky * 1536
                    src = aw[:, n0 : n0 + 1536].rearrange(
                        "(c ss) (ci8 r) -> c ss ci8 r", c=4, ci8=8
                    )
                    eng.dma_start(
                        wq[p0 : p0 + 32, s, b, :, :],
                        src[:, 8 * s, :, :],
                    )

                for b in range(2):
                    for ky in range(KS):
                        rearr(nc.scalar, 0, b, ky)
                        rearr(nc.sync, 1, b, ky)
                for s in (2, 3):
                    for b in range(2):
                        for ky in range(KS):
                            rearr(nc.gpsimd, s, b, ky)

                # final warm-up matmul closes the accumulation group
                nc.tensor.matmul(
                    dup[:], dumT[:, 0:128], dumT[:], start=False, stop=True
                )

            # (b,co)-interleave per kx: a pure per-partition free-dim
            # permutation -- engine copies, no DMA issue/receipt cost.
            # Sample 0's on DVE (it gates the conv start), the rest on
            # gpsimd (late deadlines, keeps DVE free for the sample-0
            # epilogues).
            for s in range(BS):
                ceng = nc.vector if s == 0 else nc.gpsimd
                for kx in range(KS):
                    ceng.tensor_copy(
                        wqf[:, s, kx, :].rearrange("p (b co) -> p b co", b=2),
                        wq[:, s, :, kx, :],
                    )

            # ---- conv: 4 samples x 2 half-samples x (3 kx x 8 banks) ----
            with tc.tile_pool(name="cps", bufs=8, space="PSUM") as cps:
                for s in range(BS):
                    obig = op.tile(
                        [128, 2, 8, 4, W], dt.bfloat16, tag="ob", name="ob"
                    )
                    for hs in range(2):
                        pss = [
                            cps.tile([128, 4, W], dt.float32, tag="ps", name="ps")
                            for _ in range(8)
                        ]
                        for kx in range(KS):
                            for gg in range(8):
                                j0 = (hs * 8 + gg) * 4
                                nc.tensor.matmul(
                                    pss[gg][:],
                                    wqf[:, s, kx, :],
                                    xq[s][:, j0 : j0 + 4, kx : kx + W],
                                    start=(kx == 0),
                                    stop=(kx == KS - 1),
                                )
                        # sample 0's epilogues run entirely on VectorE:
                        # ScalarE is still issuing the later samples'
                        # rearrange DMAs in that window.
                        hsid = s * 2 + hs
                        last_hs = hsid == 2 * BS - 1
                        for gg in range(8):
                            if s > 0 and gg % 2 == 0:
                                nc.scalar.activation(
                                    obig[:, hs, gg],
                                    pss[gg][:],
                                    Relu,
                                    bias=biasF[:, 8 * s : 8 * s + 1],
                                )
                            else:
                                nc.vector.tensor_scalar(
                                    obig[:, hs, gg],
                                    pss[gg][:],
                                    biasF[:, 8 * s : 8 * s + 1],
                                    0.0,
                                    mybir.AluOpType.add,
                                    mybir.AluOpType.max,
                                )
                            # final half-sample: drain each quarter as
                            # soon as its two banks are done (HWDGE only
                            # -- terminal receipts gate program end)
                            if last_hs and gg % 2 == 1:
                                eng = nc.scalar if (gg // 2) % 2 == 0 else nc.sync
                                eng.dma_start(
                                    outd.ap()[s, :, hs, gg - 1 : gg + 1],
                                    obig[:, hs, gg - 1 : gg + 1],
                                )
                        if s >= 2 and not last_hs:
                            # samples 2-3: per half-sample (a tail-end
                            # 2MB DMA would contend with the final
                            # quarters' terminal receipts)
                            eng = nc.gpsimd if hsid % 2 == 1 else nc.scalar
                            eng.dma_start(outd.ap()[s, :, hs], obig[:, hs])
                    if s < 2:
                        # samples 0-1: one 2MB DMA for the whole sample
                        # (better GB/s than two 1MB), alternating queues
                        eng = nc.scalar if s % 2 == 0 else nc.gpsimd
                        eng.dma_start(outd.ap()[s], obig[:])

    nc.compile()
    return nc


def _host_prep(x, z, w1, b1, w2, b2, w3, b3):
    """Build per-core input maps (np arrays, bf16 where device expects)."""
    x = np.asarray(x, np.float32)
    z = np.asarray(z, np.float32)
    w1 = np.asarray(w1, np.float32)
    b1 = np.asarray(b1, np.float32)
    w2 = np.asarray(w2, np.float32)
    b2 = np.asarray(b2, np.float32)
    w3 = np.asarray(w3, np.float32)
    b3 = np.asarray(b3, np.float32)

    w1a = np.concatenate([w1.T, b1[None, :]], axis=0)  # (65, 20)
    w2a = np.concatenate([w2.T, b2[None, :]], axis=0)  # (21, 30)
    # extra output column: 1.0 x (h1a's constant-1 bias feature) -> the
    # bias-feature row the device needs in each h2d diagonal block
    ecol = np.zeros((L1 + 1, 1), np.float32)
    ecol[L1, 0] = 1.0
    w2a = np.concatenate([w2a, ecol], axis=1)  # (21, 31)

    # w3 rows for chunk c, ordered (ky, ci8, kx, co):
    # old j = ((co*CIN + ci)*3 + ky)*3 + kx with ci = 8c + ci8
    ky = np.arange(KS)[:, None, None, None]
    ci8 = np.arange(8)[None, :, None, None]
    kx = np.arange(KS)[None, None, :, None]
    co = np.arange(COUT)[None, None, None, :]
    w3flat = np.concatenate([w3.T, b3[None, :]], axis=0)  # (L2+1, NOUT)
    # packed layout: cols 0:4 zaT (per-core, filled below), 4:24 w1a,
    # 24:55 w2a, 55:MOF pad, MOF: the w3 chunks
    w3a = np.zeros((128, MOF + NCH), np.float32)
    w3a[0 : FIN + 1, 4 : 4 + L1] = w1a
    w3a[0 : L1 + 1, 24 : 24 + L2 + 1] = w2a
    for c in range(4):
        oldj = (
            (co * CIN + (8 * c + ci8)) * KS * KS + ky * KS + kx
        ).reshape(-1)
        w3a[32 * c : 32 * c + L2 + 1, MOF : MOF + NCW] = w3flat[:, oldj]
    w3a[96 : 96 + L2 + 1, MOF + NCW : MOF + NCH] = w3flat[:, NKW:NOUT]

    HP = H + 2
    in_maps = []
    for core in range(N_CORES):
        sl = slice(core * BS, (core + 1) * BS)
        xs = x[sl].astype(BF16)  # (BS, CIN, H, W)
        xpad = np.zeros((BS, CIN, HP, XC), BF16)
        xpad[:, :, 1 : H + 1, 1 : W + 1] = xs
        # bottom half: partition (dy,ci) dy in {0,1}: rows 2j+dy
        xqb = np.zeros((BS, 64, XJ, XC), BF16)
        for dy in range(2):
            xqb[:, dy * 32 : dy * 32 + 32, :, :] = xpad[
                :, :, dy : dy + 2 * XJ : 2, :
            ]
        # host-duplicated: top half = bottom half at j+1 (all samples)
        xqf = np.zeros((BS, 128, XJ, XC), BF16)
        xqf[:, 0:64] = xqb
        xqf[:, 64:128, 0 : XJ - 1] = xqb[:, :, 1:XJ]
        zaT = np.concatenate(
            [z[sl].T, np.ones((1, BS), np.float32)], axis=0
        )  # (65, BS)
        w3c = w3a.copy()
        w3c[0 : FIN + 1, 0:BS] = zaT
        in_maps.append(
            {
                "xa0": xqf[0],
                "xa123": xqf[1:],
                "w3a": w3c.astype(BF16),
            }
        )
    return in_maps


_NC_CACHE = {}
LAST_EXEC_NS = None
LAST_TRACE_DIR = None


def _get_nc():
    if "nc" not in _NC_CACHE:
        _NC_CACHE["nc"] = build_nc()
    return _NC_CACHE["nc"]


def kernel(x, z, w1, b1, w2, b2, w3, b3, _trace=False):
    global LAST_EXEC_NS, LAST_TRACE_DIR
    _install_ntff_hook()
    from concourse.bass_utils import run_bass_kernel_spmd

    nc = _get_nc()
    in_maps = _host_prep(x, z, w1, b1, w2, b2, w3, b3)
    kwargs = {}
    if _trace:
        import tempfile

        LAST_TRACE_DIR = tempfile.mkdtemp(prefix="adaptconv_trace_")
        kwargs = dict(trace=True, tmpdir=LAST_TRACE_DIR)
    res = run_bass_kernel_spmd(
        nc, in_maps, core_ids=list(range(N_CORES)), **kwargs
    )
    LAST_EXEC_NS = res.exec_time_ns
    cores = []
    for i in range(N_CORES):
        arr = np.asarray(res.results[i]["out"])  # (BS, 128, 2, 8, 4, W) bf16
        y = (
            arr.reshape(BS, 2, COUT, 2, 8, 4, W)  # s, b, co, hs, gg, rp, c
            .transpose(0, 2, 3, 4, 5, 1, 6)  # s, co, hs, gg, rp, b, c
            .reshape(BS, COUT, H, W)
        )
        cores.append(y)
    return np.concatenate(cores, axis=0).astype(np.float32)
